# revision 1
# baseline (speedup 1.0000x reference)
"""AnswerHead kernel for 8 TRN2 NeuronCores.

reference:  VC = VE @ W.T + b ; out[l,b,t,v] = einsum('lbtd,vd->lbtv', A, VC)

Reassociated:  logits = (A @ W) @ VE.T + (A @ b)[:, None]
  - cuts FLOPs from ~65G to ~30G (contract A with W first: A is [640, D],
    not [V, D])
  - V is sharded across the 8 cores (tensor parallel over vocab logits),
    A/W/b replicated; each core emits a [640, V/8] logit slab, host concat.

Device work per core:
  warmup : data-independent matmuls so the PE HAM clock-gate is at 8/8
           before real work arrives
  phase 1: T^T[k, n] = sum_d W[d, k] * A^T[d, n]           (PE, 72 matmuls)
           ab[n]     = sum_d A[n, d] * b[d]                (PE, 30 matmuls)
  phase 2: out[n, v] = sum_k T^T[k, n] * VET[k, v] + ab[n] (PE, 300 matmuls,
           bias fused into the PSUM->SBUF copy on VectorE)

All host work is layout-only (transpose / cast / slice) — every FLOP is on
device.  Inputs are pre-shuffled on host into partition-major SBUF images so
each DMA descriptor is a multi-KB contiguous run.  W is fed in k-major
chunks so phase 1's first kc-group depends on `at` plus only 1/6 of W — the
PE, not the DMA stream, then paces phase 1.  Inputs stream on the sync
HWDGE ring, outputs (bf16) on the scalar (ACT) HWDGE ring so reads and
writes don't share a FIFO.
Compute dtype bf16 (PE runs fp32 at 1/4 rate; rel-err gate is 2e-2).
"""

import sys

if "/opt/trn_rl_repo" not in sys.path:
    sys.path.insert(0, "/opt/trn_rl_repo")

import numpy as np
import ml_dtypes

L, B, T, D, V = 2, 16, 20, 768, 30000
N = L * B * T            # 640 tokens
NCORES = 8
VS = V // NCORES         # 3750 vocab rows per core
P = 128
DC = D // P              # 6 contraction chunks of 128
KC = D // P              # 6 output-k chunks of 128 (phase 1)
NCH = N // P             # 5 token chunks of 128
G = 10                   # v groups per core
VG = VS // G             # 375 logits per group (fits one PSUM bank in f32)
N_WARM = 20              # warmup matmuls: bridge PE activity until inputs land

BF16 = ml_dtypes.bfloat16

_TRACE = False
_TRACE_KW = {}
LAST = {}
_cache = {}


def _build():
    import concourse.mybir as mybir
    import concourse.tile as tile
    from concourse import bacc

    nc = bacc.Bacc(
        "TRN2", target_bir_lowering=False, debug=False, num_devices=NCORES
    )
    bf = mybir.dt.bfloat16
    f32 = mybir.dt.float32
    add = mybir.AluOpType.add

    at_d = nc.declare_dram_parameter("at", [P, DC, N], bf, isOutput=False)
    w_d = nc.declare_dram_parameter("w", [P, KC, DC, P], bf, isOutput=False)
    b_d = nc.declare_dram_parameter("bvec", [P, DC], bf, isOutput=False)
    vet_d = nc.declare_dram_parameter("vet", [P, G, DC, VG], bf, isOutput=False)
    out_d = nc.declare_dram_parameter("out", [G, P, NCH, VG], bf, isOutput=True)

    with tile.TileContext(nc) as tc:
        with (
            tc.tile_pool(name="const", bufs=1) as cpool,
            tc.tile_pool(name="outp", bufs=6) as opool,
            tc.tile_pool(name="ps1", bufs=2, space="PSUM") as ps1,
            tc.tile_pool(name="ps2", bufs=6, space="PSUM") as ps2,
        ):
            at_sb = cpool.tile([P, DC, N], bf, tag="at")
            w_sb = cpool.tile([P, KC, DC, P], bf, tag="w")
            b_sb = cpool.tile([P, DC], bf, tag="b")
            vet_sb = cpool.tile([P, G, DC, VG], bf, tag="vet")
            tt_sb = cpool.tile([P, KC, N], bf, tag="tt")
            ab_sb = cpool.tile([P, NCH], f32, tag="ab")
            warm_sb = cpool.tile([P, 640], bf, tag="warm")

            # ---- PE warmup: data-independent matmuls to lift the HAM
            # clock gate to 8/8 while input DMAs are in flight.
            nc.gpsimd.memset(warm_sb[:], 0.0)
            for i in range(N_WARM):
                pool = ps1 if i % 2 == 0 else ps2
                wps = pool.tile([P, 512], f32, tag=pool.name)
                nc.tensor.matmul(
                    wps[:], warm_sb[:, :P], warm_sb[:, P : P + 512]
                )

            # ---- input DMAs (sync ring), critical-path first:
            # at (everything needs it), then W in kc chunks (phase 1 follows
            # them), then b, then vet groups (phase 2 follows them).
            for dc in range(DC):
                nc.sync.dma_start(at_sb[:, dc, :], at_d.ap()[:, dc, :])
            for kc in range(KC):
                nc.sync.dma_start(w_sb[:, kc], w_d.ap()[:, kc])
            nc.sync.dma_start(b_sb[:], b_d.ap())
            for g in range(G):
                nc.sync.dma_start(vet_sb[:, g], vet_d.ap()[:, g])

            # ---- phase 1: T^T[k, n]  (k on partitions, per 128-chunk);
            # group kc depends on at + w[:, kc] only.
            for kc in range(KC):
                for idx, (n0, nl) in enumerate(((0, 320), (320, 320))):
                    pool = ps1 if (2 * kc + idx) % 2 == 0 else ps2
                    ps = pool.tile([P, 512], f32, tag=pool.name)
                    for dc in range(DC):
                        nc.tensor.matmul(
                            ps[:, :nl],
                            w_sb[:, kc, dc, :],
                            at_sb[:, dc, n0 : n0 + nl],
                            start=(dc == 0),
                            stop=(dc == DC - 1),
                        )
                    nc.vector.tensor_copy(tt_sb[:, kc, n0 : n0 + nl], ps[:, :nl])

            # ---- ab[n] = sum_d A[n,d] b[d], laid out [128, NCH]
            for ni in range(NCH):
                ps = ps1.tile([P, 512], f32, tag="ps1")
                for dc in range(DC):
                    nc.tensor.matmul(
                        ps[:, :1],
                        at_sb[:, dc, ni * P : (ni + 1) * P],
                        b_sb[:, dc, None],
                        start=(dc == 0),
                        stop=(dc == DC - 1),
                    )
                nc.vector.tensor_copy(ab_sb[:, ni : ni + 1], ps[:, :1])

            # ---- phase 2: logits[n, v] = T^T.T @ VET + ab
            for g in range(G):
                last_g = g == G - 1
                ot = opool.tile([P, NCH, VG], bf, tag="ot")
                for ni in range(NCH):
                    ps = ps2.tile([P, 512], f32, tag="ps2")
                    for kc in range(KC):
                        nc.tensor.matmul(
                            ps[:, :VG],
                            tt_sb[:, kc, ni * P : (ni + 1) * P],
                            vet_sb[:, g, kc, :],
                            start=(kc == 0),
                            stop=(kc == KC - 1),
                        )
                    # out = psum + ab (per-partition bias) on VectorE, cast bf16
                    nc.vector.tensor_tensor(
                        ot[:, ni, :],
                        ps[:, :VG],
                        ab_sb[:, ni, None].to_broadcast((P, VG)),
                        add,
                    )
                    if last_g:
                        # final group: ship each slice as soon as its copy
                        # lands, so the tail isn't one big trailing DMA
                        nc.scalar.dma_start(
                            out_d.ap()[g, :, ni, :], ot[:, ni, :]
                        )
                if not last_g:
                    # one fat out DMA per group on the ACT HWDGE ring
                    nc.scalar.dma_start(out_d.ap()[g], ot[:])

    nc.compile()
    return nc


def _get_nc():
    if "nc" not in _cache:
        _cache["nc"] = _build()
    return _cache["nc"]


def kernel(answer_embed, vocab_embed, W, b):
    from concourse.bass_utils import run_bass_kernel_spmd

    answer_embed = np.asarray(answer_embed, dtype=np.float32)
    vocab_embed = np.asarray(vocab_embed, dtype=np.float32)
    W = np.asarray(W, dtype=np.float32)
    b = np.asarray(b, dtype=np.float32)

    A = answer_embed.reshape(N, D)
    # partition-major images: index [p, c, ...] maps to dim value c*128+p
    at = A.reshape(N, DC, P).transpose(2, 1, 0).astype(BF16)       # [P,DC,N]
    # W image [p, kc, dc, kcol]: W[dc*128+p, kc*128+kcol]
    w = W.reshape(DC, P, KC, P).transpose(1, 2, 0, 3).astype(BF16)
    bv = b.reshape(DC, P).T.astype(BF16)                           # [P,DC]

    in_maps = []
    for i in range(NCORES):
        ve_i = vocab_embed[i * VS : (i + 1) * VS]                  # [VS, D]
        vet = ve_i.reshape(G, VG, DC, P).transpose(3, 0, 2, 1).astype(BF16)
        in_maps.append({"at": at, "w": w, "bvec": bv, "vet": vet})

    nc = _get_nc()
    res = run_bass_kernel_spmd(
        nc, in_maps, core_ids=list(range(NCORES)), **(_TRACE_KW if _TRACE else {})
    )
    if _TRACE:
        LAST["exec_time_ns"] = res.exec_time_ns
        LAST["results"] = res

    # out[g, p, ni, v] -> logits[ni*128+p, g*VG+v]
    slabs = [
        res.results[i]["out"].astype(np.float32).transpose(2, 1, 0, 3).reshape(N, VS)
        for i in range(NCORES)
    ]
    full = np.concatenate(slabs, axis=1)
    return full.reshape(L, B, T, V).astype(np.float32)



# revision 2
# speedup vs baseline: 1.1866x; 1.1866x over previous
"""AnswerHead kernel for 8 TRN2 NeuronCores.

reference:  VC = VE @ W.T + b ; out[l,b,t,v] = einsum('lbtd,vd->lbtv', A, VC)

Reassociated:  logits = (A @ W) @ VE.T + (A @ b)[:, None]
  - cuts FLOPs from ~65G to ~30G (contract A with W first: A is [640, D],
    not [V, D])
  - V is sharded across the 8 cores (tensor parallel over vocab logits),
    A/W/b replicated; each core emits a [640, V/8] logit slab, host concat.

Device work per core:
  warmup : data-independent matmuls so the PE HAM clock-gate ramps while
           the first input DMAs are in flight (~3.4us ramp window)
  phase 1: T^T[k, n] = sum_d W[d, k] * A^T[d, n]           (PE)
           ab[n]     = sum_d A[n, d] * b[d]                (PE)
  phase 2: out[n, v] = sum_k T^T[k, n] * VET[k, v] + ab[n] (PE, 300 matmuls,
           bias fused into the PSUM->SBUF copy on VectorE)

All host work is layout-only (transpose / cast / slice) — every FLOP is on
device.  Inputs are pre-shuffled on host into partition-major SBUF images so
each DMA descriptor is a multi-KB contiguous run.  DMA triggers cost ~0.7us
each on the sync queue regardless of size, so `at` ships as two fat
triggers (384- and 256-token chunks) and W in k-major chunks right behind:
phase 1's first kc-group depends on at0 + w[:,0] only, and starts ~10us in
instead of ~16.5us.  Inputs stream on the sync HWDGE ring, outputs (bf16)
on the scalar (ACT) HWDGE ring so reads and writes don't share a FIFO.
Compute dtype bf16 (PE runs fp32 at 1/4 rate; rel-err gate is 2e-2).
"""

import sys

if "/opt/trn_rl_repo" not in sys.path:
    sys.path.insert(0, "/opt/trn_rl_repo")

import numpy as np
import ml_dtypes

L, B, T, D, V = 2, 16, 20, 768, 30000
N = L * B * T            # 640 tokens
NCORES = 8
VS = V // NCORES         # 3750 vocab rows per core
P = 128
DC = D // P              # 6 contraction chunks of 128
KC = D // P              # 6 output-k chunks of 128 (phase 1)
NCH = N // P             # 5 token chunks of 128
G = 10                   # v groups per core
VG = VS // G             # 375 logits per group (fits one PSUM bank in f32)
N0, N1 = 384, 256        # token split: 3 + 2 chunks of 128
N_WARM = 5               # warmup matmuls: bridge PE HAM ramp until inputs land
VETB = 2                 # vet groups per DMA trigger

BF16 = ml_dtypes.bfloat16

_TRACE = False
_TRACE_KW = {}
LAST = {}
_cache = {}


def _build():
    import concourse.mybir as mybir
    import concourse.tile as tile
    from concourse import bacc

    nc = bacc.Bacc(
        "TRN2", target_bir_lowering=False, debug=False, num_devices=NCORES
    )
    bf = mybir.dt.bfloat16
    f32 = mybir.dt.float32
    add = mybir.AluOpType.add

    at0_d = nc.declare_dram_parameter("at0", [P, DC, N0], bf, isOutput=False)
    at1_d = nc.declare_dram_parameter("at1", [P, DC, N1], bf, isOutput=False)
    w_d = nc.declare_dram_parameter("w", [P, KC, DC, P], bf, isOutput=False)
    b_d = nc.declare_dram_parameter("bvec", [P, DC], bf, isOutput=False)
    vet_d = nc.declare_dram_parameter("vet", [P, G, DC, VG], bf, isOutput=False)
    out_d = nc.declare_dram_parameter("out", [G, P, NCH, VG], bf, isOutput=True)

    with tile.TileContext(nc) as tc:
        with (
            tc.tile_pool(name="const", bufs=1) as cpool,
            tc.tile_pool(name="outp", bufs=6) as opool,
            tc.tile_pool(name="ps1", bufs=2, space="PSUM") as ps1,
            tc.tile_pool(name="ps2", bufs=6, space="PSUM") as ps2,
        ):
            at0_sb = cpool.tile([P, DC, N0], bf, tag="at0")
            at1_sb = cpool.tile([P, DC, N1], bf, tag="at1")
            w_sb = cpool.tile([P, KC, DC, P], bf, tag="w")
            b_sb = cpool.tile([P, DC], bf, tag="b")
            vet_sb = cpool.tile([P, G, DC, VG], bf, tag="vet")
            tt_sb = cpool.tile([P, KC, N], bf, tag="tt")
            ab_sb = cpool.tile([P, NCH], f32, tag="ab")
            warm_sb = cpool.tile([P, 640], bf, tag="warm")

            # ---- input DMAs (sync ring), critical-path first: at0, w0
            # unblock phase 1's first kc-group; each trigger costs ~0.7us of
            # sync-queue issue time, so at ships as 2 fat triggers.
            nc.sync.dma_start(at0_sb[:], at0_d.ap())
            nc.sync.dma_start(w_sb[:, 0], w_d.ap()[:, 0])
            nc.sync.dma_start(at1_sb[:], at1_d.ap())
            for kc in range(1, KC):
                nc.sync.dma_start(w_sb[:, kc], w_d.ap()[:, kc])
            nc.sync.dma_start(b_sb[:], b_d.ap())
            for g0 in range(0, G, VETB):
                nc.sync.dma_start(
                    vet_sb[:, g0 : g0 + VETB], vet_d.ap()[:, g0 : g0 + VETB]
                )

            # ---- PE warmup: data-independent matmuls to lift the HAM
            # clock gate while the input DMAs stream.
            nc.gpsimd.memset(warm_sb[:], 0.0)
            for i in range(N_WARM):
                pool = ps1 if i % 2 == 0 else ps2
                wps = pool.tile([P, 512], f32, tag=pool.name)
                nc.tensor.matmul(
                    wps[:], warm_sb[:, :P], warm_sb[:, P : P + 512]
                )

            # ---- phase 1: T^T[k, n]  (k on partitions, per 128-chunk);
            # group (kc, h) depends on at<h> + w[:, kc] only.
            for kc in range(KC):
                for idx, (src, n0, nl) in enumerate(
                    ((at0_sb, 0, N0), (at1_sb, N0, N1))
                ):
                    pool = ps1 if (2 * kc + idx) % 2 == 0 else ps2
                    ps = pool.tile([P, 512], f32, tag=pool.name)
                    for dc in range(DC):
                        nc.tensor.matmul(
                            ps[:, :nl],
                            w_sb[:, kc, dc, :],
                            src[:, dc, :],
                            start=(dc == 0),
                            stop=(dc == DC - 1),
                        )
                    nc.vector.tensor_copy(tt_sb[:, kc, n0 : n0 + nl], ps[:, :nl])

            # ---- ab[n] = sum_d A[n,d] b[d], laid out [128, NCH]
            for ni in range(NCH):
                src, off = (at0_sb, 0) if ni < 3 else (at1_sb, 3)
                c0 = (ni - off) * P
                ps = ps1.tile([P, 512], f32, tag="ps1")
                for dc in range(DC):
                    nc.tensor.matmul(
                        ps[:, :1],
                        src[:, dc, c0 : c0 + P],
                        b_sb[:, dc, None],
                        start=(dc == 0),
                        stop=(dc == DC - 1),
                    )
                nc.vector.tensor_copy(ab_sb[:, ni : ni + 1], ps[:, :1])

            # ---- phase 2: logits[n, v] = T^T.T @ VET + ab
            for g in range(G):
                last_g = g == G - 1
                ot = opool.tile([P, NCH, VG], bf, tag="ot")
                for ni in range(NCH):
                    ps = ps2.tile([P, 512], f32, tag="ps2")
                    for kc in range(KC):
                        nc.tensor.matmul(
                            ps[:, :VG],
                            tt_sb[:, kc, ni * P : (ni + 1) * P],
                            vet_sb[:, g, kc, :],
                            start=(kc == 0),
                            stop=(kc == KC - 1),
                        )
                    # out = psum + ab (per-partition bias) on VectorE, cast bf16
                    nc.vector.tensor_tensor(
                        ot[:, ni, :],
                        ps[:, :VG],
                        ab_sb[:, ni, None].to_broadcast((P, VG)),
                        add,
                    )
                    if last_g:
                        # final group: ship each slice as soon as its copy
                        # lands, so the tail isn't one big trailing DMA
                        nc.scalar.dma_start(
                            out_d.ap()[g, :, ni, :], ot[:, ni, :]
                        )
                if not last_g:
                    # one fat out DMA per group on the ACT HWDGE ring
                    nc.scalar.dma_start(out_d.ap()[g], ot[:])

    nc.compile()
    return nc


def _get_nc():
    if "nc" not in _cache:
        _cache["nc"] = _build()
    return _cache["nc"]


def kernel(answer_embed, vocab_embed, W, b):
    from concourse.bass_utils import run_bass_kernel_spmd

    answer_embed = np.asarray(answer_embed, dtype=np.float32)
    vocab_embed = np.asarray(vocab_embed, dtype=np.float32)
    W = np.asarray(W, dtype=np.float32)
    b = np.asarray(b, dtype=np.float32)

    A = answer_embed.reshape(N, D)
    # partition-major images: index [p, c, ...] maps to dim value c*128+p
    at = A.reshape(N, DC, P).transpose(2, 1, 0).astype(BF16)       # [P,DC,N]
    at0 = np.ascontiguousarray(at[:, :, :N0])
    at1 = np.ascontiguousarray(at[:, :, N0:])
    # W image [p, kc, dc, kcol]: W[dc*128+p, kc*128+kcol]
    w = W.reshape(DC, P, KC, P).transpose(1, 2, 0, 3).astype(BF16)
    bv = b.reshape(DC, P).T.astype(BF16)                           # [P,DC]

    in_maps = []
    for i in range(NCORES):
        ve_i = vocab_embed[i * VS : (i + 1) * VS]                  # [VS, D]
        vet = ve_i.reshape(G, VG, DC, P).transpose(3, 0, 2, 1).astype(BF16)
        in_maps.append({"at0": at0, "at1": at1, "w": w, "bvec": bv, "vet": vet})

    nc = _get_nc()
    res = run_bass_kernel_spmd(
        nc, in_maps, core_ids=list(range(NCORES)), **(_TRACE_KW if _TRACE else {})
    )
    if _TRACE:
        LAST["exec_time_ns"] = res.exec_time_ns
        LAST["results"] = res

    # out[g, p, ni, v] -> logits[ni*128+p, g*VG+v]
    slabs = [
        res.results[i]["out"].astype(np.float32).transpose(2, 1, 0, 3).reshape(N, VS)
        for i in range(NCORES)
    ]
    full = np.concatenate(slabs, axis=1)
    return full.reshape(L, B, T, V).astype(np.float32)


# revision 8
# speedup vs baseline: 1.2170x; 1.0256x over previous
"""AnswerHead kernel for 8 TRN2 NeuronCores.

reference:  VC = VE @ W.T + b ; out[l,b,t,v] = einsum('lbtd,vd->lbtv', A, VC)

Reassociated:  logits = (A @ W) @ VE.T + (A @ b)[:, None]
  - cuts FLOPs from ~65G to ~30G (contract A with W first: A is [640, D],
    not [V, D])
  - V is sharded across the 8 cores (tensor parallel over vocab logits),
    A/W/b replicated; each core emits a [640, V/8] logit slab, host concat.

Device work per core:
  warmup : data-independent matmuls so the PE HAM clock-gate ramps while
           the first input DMAs are in flight (~3.4us ramp window)
  phase 1: T^T[k, n] = sum_d W[d, k] * A^T[d, n]           (PE)
           ab[n]     = sum_d A[n, d] * b[d]                (PE)
  phase 2: out[n, v] = sum_k T^T[k, n] * VET[k, v] + ab[n] (PE, 300 matmuls,
           bias fused into the PSUM->SBUF copy on VectorE)

All host work is layout-only (transpose / cast / slice) — every FLOP is on
device.  Inputs are pre-shuffled on host into partition-major SBUF images so
each DMA descriptor is a multi-KB contiguous run.  DMA triggers cost ~0.7us
each on the sync queue regardless of size, so `at` ships as two fat
triggers (384- and 256-token chunks) and W in k-major chunks right behind:
phase 1's first kc-group depends on at0 + w[:,0] only, and starts ~10us in
instead of ~16.5us.  Inputs stream on the sync HWDGE ring, outputs (bf16)
on the scalar (ACT) HWDGE ring so reads and writes don't share a FIFO.
Compute dtype bf16 (PE runs fp32 at 1/4 rate; rel-err gate is 2e-2).
"""

import sys

if "/opt/trn_rl_repo" not in sys.path:
    sys.path.insert(0, "/opt/trn_rl_repo")

import numpy as np
import ml_dtypes

L, B, T, D, V = 2, 16, 20, 768, 30000
N = L * B * T            # 640 tokens
NCORES = 8
VS = V // NCORES         # 3750 vocab rows per core
P = 128
DC = D // P              # 6 contraction chunks of 128
KC = D // P              # 6 output-k chunks of 128 (phase 1)
NCH = N // P             # 5 token chunks of 128
G = 10                   # v groups per core
VG = VS // G             # 375 logits per group (fits one PSUM bank in f32)
N_WARM = 2               # warmup matmuls: bridge PE HAM ramp until inputs land
VETB = 2                 # vet groups per DMA trigger

# phase-1 unit (kc, c) execution order, matched to the DMA arrival order of
# w[kc] / at[c] below (w0,c0,c1,w1,c2,w2,c3,c4,w3,w4,w5) so the PE is never
# waiting on a transfer that was triggered later than one it already consumed.
P1_SEQ = [
    (0, 0), (0, 1), (1, 0), (1, 1), (0, 2), (1, 2),
    (2, 0), (2, 1), (2, 2), (0, 3), (1, 3), (2, 3),
    (3, 0), (3, 1), (3, 2), (3, 3), (0, 4), (1, 4), (2, 4), (3, 4),
    (4, 0), (4, 1), (4, 2), (4, 3), (4, 4),
    (5, 0), (5, 1), (5, 2), (5, 3), (5, 4),
]

BF16 = ml_dtypes.bfloat16

_TRACE = False
_TRACE_KW = {}
LAST = {}
_cache = {}


def _build():
    import concourse.mybir as mybir
    import concourse.tile as tile
    from concourse import bacc

    nc = bacc.Bacc(
        "TRN2", target_bir_lowering=False, debug=False, num_devices=NCORES
    )
    bf = mybir.dt.bfloat16
    f32 = mybir.dt.float32
    add = mybir.AluOpType.add

    at_d = nc.declare_dram_parameter("at", [P, NCH, DC, P], bf, isOutput=False)
    w_d = nc.declare_dram_parameter("w", [P, KC, DC, P], bf, isOutput=False)
    b_d = nc.declare_dram_parameter("bvec", [P, DC], bf, isOutput=False)
    vet_d = nc.declare_dram_parameter("vet", [P, G, DC, VG], bf, isOutput=False)
    out_d = nc.declare_dram_parameter("out", [G, P, NCH, VG], bf, isOutput=True)

    with tile.TileContext(nc) as tc:
        with (
            tc.tile_pool(name="const", bufs=1) as cpool,
            tc.tile_pool(name="outp", bufs=6) as opool,
            tc.tile_pool(name="ps1", bufs=2, space="PSUM") as ps1,
            tc.tile_pool(name="ps2", bufs=6, space="PSUM") as ps2,
        ):
            at_sb = cpool.tile([P, NCH, DC, P], bf, tag="at")
            w_sb = cpool.tile([P, KC, DC, P], bf, tag="w")
            b_sb = cpool.tile([P, DC], bf, tag="b")
            vet_sb = cpool.tile([P, G, DC, VG], bf, tag="vet")
            tt_sb = cpool.tile([P, KC, N], bf, tag="tt")
            ab_sb = cpool.tile([P, NCH], f32, tag="ab")
            warm_sb = cpool.tile([P, 640], bf, tag="warm")

            # ---- input DMAs (sync ring), arrival order matched to P1_SEQ;
            # each trigger costs ~0.7us of sync-queue issue time, so at/w
            # ship as 196KB chunk triggers in consumption order.
            def w_trig(kc):
                nc.sync.dma_start(w_sb[:, kc], w_d.ap()[:, kc])

            def at_trig(c):
                nc.sync.dma_start(at_sb[:, c], at_d.ap()[:, c])

            w_trig(0); at_trig(0); at_trig(1); w_trig(1); at_trig(2)
            w_trig(2); at_trig(3); at_trig(4); w_trig(3); w_trig(4); w_trig(5)
            nc.sync.dma_start(b_sb[:], b_d.ap())
            for g0 in range(0, G, VETB):
                nc.sync.dma_start(
                    vet_sb[:, g0 : g0 + VETB], vet_d.ap()[:, g0 : g0 + VETB]
                )

            # ---- PE warmup: data-independent matmuls to lift the HAM
            # clock gate while the input DMAs stream.
            nc.gpsimd.memset(warm_sb[:], 0.0)
            for i in range(N_WARM):
                pool = ps1 if i % 2 == 0 else ps2
                wps = pool.tile([P, 512], f32, tag=pool.name)
                nc.tensor.matmul(
                    wps[:], warm_sb[:, :P], warm_sb[:, P : P + 512]
                )

            # ---- phase 1: T^T[k, n]  (k on partitions, per 128-chunk);
            # unit (kc, c) depends on w[:, kc] + at[:, c] only, ordered so
            # the PE consumes transfers in the order they land.
            for idx, (kc, c) in enumerate(P1_SEQ):
                pool = ps1 if idx % 2 == 0 else ps2
                ps = pool.tile([P, 512], f32, tag=pool.name)
                for dc in range(DC):
                    nc.tensor.matmul(
                        ps[:, :P],
                        w_sb[:, kc, dc, :],
                        at_sb[:, c, dc, :],
                        start=(dc == 0),
                        stop=(dc == DC - 1),
                    )
                nc.vector.tensor_copy(tt_sb[:, kc, c * P : (c + 1) * P], ps[:, :P])

            # ---- ab[n] = sum_d A[n,d] b[d], laid out [128, NCH]
            for ni in range(NCH):
                ps = ps1.tile([P, 512], f32, tag="ps1")
                for dc in range(DC):
                    nc.tensor.matmul(
                        ps[:, :1],
                        at_sb[:, ni, dc, :],
                        b_sb[:, dc, None],
                        start=(dc == 0),
                        stop=(dc == DC - 1),
                    )
                nc.vector.tensor_copy(ab_sb[:, ni : ni + 1], ps[:, :1])

            # ---- phase 2: logits[n, v] = T^T.T @ VET + ab
            for g in range(G):
                last_g = g == G - 1
                ot = opool.tile([P, NCH, VG], bf, tag="ot")
                for ni in range(NCH):
                    ps = ps2.tile([P, 512], f32, tag="ps2")
                    for kc in range(KC):
                        nc.tensor.matmul(
                            ps[:, :VG],
                            tt_sb[:, kc, ni * P : (ni + 1) * P],
                            vet_sb[:, g, kc, :],
                            start=(kc == 0),
                            stop=(kc == KC - 1),
                        )
                    # out = psum + ab (per-partition bias) on VectorE, cast bf16
                    nc.vector.tensor_tensor(
                        ot[:, ni, :],
                        ps[:, :VG],
                        ab_sb[:, ni, None].to_broadcast((P, VG)),
                        add,
                    )
                    if last_g:
                        # final group: ship each slice as soon as its copy
                        # lands, so the tail isn't one big trailing DMA
                        nc.scalar.dma_start(
                            out_d.ap()[g, :, ni, :], ot[:, ni, :]
                        )
                if not last_g:
                    # one fat out DMA per group on the ACT HWDGE ring
                    nc.scalar.dma_start(out_d.ap()[g], ot[:])

    nc.compile()
    return nc


def _get_nc():
    if "nc" not in _cache:
        _cache["nc"] = _build()
    return _cache["nc"]


def kernel(answer_embed, vocab_embed, W, b):
    from concourse.bass_utils import run_bass_kernel_spmd

    answer_embed = np.asarray(answer_embed, dtype=np.float32)
    vocab_embed = np.asarray(vocab_embed, dtype=np.float32)
    W = np.asarray(W, dtype=np.float32)
    b = np.asarray(b, dtype=np.float32)

    A = answer_embed.reshape(N, D)
    # at image [p, c, dc, j]: A[c*128+j, dc*128+p]  (token chunks of 128)
    at = A.reshape(NCH, P, DC, P).transpose(3, 0, 2, 1).astype(BF16)
    # W image [p, kc, dc, kcol]: W[dc*128+p, kc*128+kcol]
    w = W.reshape(DC, P, KC, P).transpose(1, 2, 0, 3).astype(BF16)
    bv = b.reshape(DC, P).T.astype(BF16)                           # [P,DC]

    in_maps = []
    for i in range(NCORES):
        ve_i = vocab_embed[i * VS : (i + 1) * VS]                  # [VS, D]
        vet = ve_i.reshape(G, VG, DC, P).transpose(3, 0, 2, 1).astype(BF16)
        in_maps.append({"at": at, "w": w, "bvec": bv, "vet": vet})

    nc = _get_nc()
    res = run_bass_kernel_spmd(
        nc, in_maps, core_ids=list(range(NCORES)), **(_TRACE_KW if _TRACE else {})
    )
    if _TRACE:
        LAST["exec_time_ns"] = res.exec_time_ns
        LAST["results"] = res

    # out[g, p, ni, v] -> logits[ni*128+p, g*VG+v]
    slabs = [
        res.results[i]["out"].astype(np.float32).transpose(2, 1, 0, 3).reshape(N, VS)
        for i in range(NCORES)
    ]
    full = np.concatenate(slabs, axis=1)
    return full.reshape(L, B, T, V).astype(np.float32)
